# revision 35
# baseline (speedup 1.0000x reference)
"""Multi-head attention (B=4, S=2048, D=1024, H=16) on 8 trn2 NeuronCores.

Sharding: (batch, head-half) -> one core each. Core c handles batch c//2 and
heads (c%2)*8 .. (c%2)*8+7 (feature columns (c%2)*512 .. +512 of the QKV
projections, rows of Wo). Each core computes its 8 heads' attention and a
partial output projection; the host sums the two partials per batch and adds
the output bias.

Device layout per core (S=2048 tokens, F=512 local features, hd=64):
  - inputs Q/K/V arrive host-transposed as [1024, 2048] so the d_model
    contraction sits on SBUF partitions,
  - q^T/k^T are produced feature-major ([512, 2048]) via lhsT=W, rhs=X^T,
  - scores are computed transposed (S^T[k, q]) so the P@V matmul can use v
    in natural [token, feature] layout as the stationary operand,
  - softmax: exp on the ACT engine with the 1/8 scale folded in; the
    denominator comes from an all-ones 65th column appended to v; the
    normalization multiplies o'^T rows by a PE-broadcast reciprocal row.

Schedule (v2): the kernel is ACT-bound in steady state (33.5M exps/core @
153.6 G/s ~= 220us floor), so attention group (0,0) starts as soon as the
kT tiles covering its first scores plus q(0,0) exist (~12us in, vs a ~150us
serial projection phase in v1). All remaining projection work (kT pieces,
v tiles, later q chunks) chases into the attention stream as deadline-driven
or filler work; chunks 1-3 are ACT-bound with PE ~85% busy. Softmax
denominators are gathered per head pair and inverted with the fast
approximate reciprocal (51 ULP) instead of an 8-cycle/elem DVE reciprocal
on a broadcast tile. Matmuls run in bf16 (fp32 PSUM accumulation).

PSUM banks: 4 scores (2 double-buffered [128,1024] tiles) + 2 PV
accumulators + 1 projection accumulator + 1 misc (broadcasts / Wo) = 8.
"""
import numpy as np

import concourse.bass as bass
import concourse.tile as tile
from concourse import mybir
from concourse.bass_utils import run_bass_kernel_spmd

F32 = mybir.dt.float32
F32R = mybir.dt.float32r
BF16 = mybir.dt.bfloat16
EXP = mybir.ActivationFunctionType.Exp

B, S, DM, H_TOT = 4, 2048, 1024, 16
F = 512          # features per core (8 heads x 64)
HD = 64          # head dim
NH = 8           # heads per core
NP = 4           # head pairs per core
KT = 16          # k tiles of 128
NQT = 4          # q chunks of 512
SCALE = 0.125    # 1/sqrt(64)
N_CORES = 8

_WAIT_CAP = {"InstEventSemaphore": 2}


def _split_multiwaits(nc):
    """This walrus build accepts 1 sync-wait per instruction (2 on
    EventSemaphore); spread extras over same-engine NOPs placed before."""
    n_spill = 0
    for f in nc.m.functions:
        for bb in f.blocks:
            new = []
            changed = False
            for inst in bb.instructions:
                si = inst.sync_info
                cap = _WAIT_CAP.get(type(inst).__name__, 1)
                if si is not None and len(si.on_wait) > cap:
                    extra = list(si.on_wait[: len(si.on_wait) - cap])
                    del si.on_wait[: len(si.on_wait) - cap]
                    for w in extra:
                        n_spill += 1
                        nop = mybir.InstNoOp(name=f"I-wspill-{n_spill}-{inst.name}")
                        nop.engine = inst.engine
                        nop.sync_info = mybir.SyncInfo(on_wait=[w], on_update=[])
                        new.append(nop)
                    changed = True
                new.append(inst)
            if changed:
                bb.instructions[:] = new
    return n_spill


def build_program():
    nc = bass.Bass("TRN2", target_bir_lowering=False, debug=False, num_devices=1)

    d_qt = nc.dram_tensor("qt", [DM, S], BF16, kind="ExternalInput").ap()
    d_kt = nc.dram_tensor("kt", [DM, S], BF16, kind="ExternalInput").ap()
    d_vt = nc.dram_tensor("vt", [DM, S], BF16, kind="ExternalInput").ap()
    d_wq = nc.dram_tensor("wq", [DM, F], BF16, kind="ExternalInput").ap()
    d_wk = nc.dram_tensor("wk", [DM, F], BF16, kind="ExternalInput").ap()
    d_wv = nc.dram_tensor("wv", [DM, F], BF16, kind="ExternalInput").ap()
    d_wo = nc.dram_tensor("wo", [F, DM], BF16, kind="ExternalInput").ap()
    d_bq = nc.dram_tensor("bq", [F], F32, kind="ExternalInput").ap()
    d_bk = nc.dram_tensor("bk", [F], F32, kind="ExternalInput").ap()
    d_bv = nc.dram_tensor("bv", [F], F32R, kind="ExternalInput").ap()
    d_ones = nc.dram_tensor("ones", [65, 128], F32R, kind="ExternalInput").ap()
    d_part = nc.dram_tensor("part", [S, DM], F32, kind="ExternalOutput").ap()

    with tile.TileContext(nc) as tc:
        with (
            tc.tile_pool(name="wpool", bufs=1) as wpool,
            tc.tile_pool(name="big", bufs=1) as big,
            tc.tile_pool(name="ktin", bufs=16) as ktin,
            tc.tile_pool(name="qtin", bufs=10) as qtin,
            tc.tile_pool(name="vtin", bufs=24) as vtin,
            tc.tile_pool(name="exch", bufs=7) as exch,
            tc.tile_pool(name="outst", bufs=2) as outst,
            tc.tile_pool(name="ocp", bufs=4) as ocp,
            tc.tile_pool(name="ps_sc", bufs=2, space="PSUM") as ps_sc,
            tc.tile_pool(name="ps_pv", bufs=2, space="PSUM") as ps_pv,
            tc.tile_pool(name="ps_proj", bufs=1, space="PSUM") as ps_proj,
            tc.tile_pool(name="ps_misc", bufs=1, space="PSUM") as ps_misc,
        ):
            # ---- resident tiles
            wq_sb = [wpool.tile([128, F], BF16, tag=f"wq{m}", name=f"wq{m}") for m in range(8)]
            wk_sb = [wpool.tile([128, F], BF16, tag=f"wk{m}", name=f"wk{m}") for m in range(8)]
            wv_sb = [wpool.tile([128, F], BF16, tag=f"wv{m}", name=f"wv{m}") for m in range(8)]
            wo_sb = [wpool.tile([128, DM], BF16, tag=f"wo{f}", name=f"wo{f}") for f in range(4)]
            qT_sb = [big.tile([128, S], BF16, tag=f"qT{f}", name=f"qT{f}") for f in range(4)]
            kT_sb = [big.tile([128, S], BF16, tag=f"kT{f}", name=f"kT{f}") for f in range(4)]
            oT_sb = [big.tile([128, S], BF16, tag=f"oT{f}", name=f"oT{f}") for f in range(4)]
            v_sb = [big.tile([128, NH * (HD + 1)], BF16, tag=f"v{t}", name=f"v{t}") for t in range(KT)]
            bq_sb = wpool.tile([128, 4], F32, tag="bq")
            bk_sb = wpool.tile([128, 4], F32, tag="bk")
            bv_sb = wpool.tile([1, F], F32R, tag="bv")
            ones_sb = wpool.tile([65, 128], F32R, tag="ones")
            bvbc_sb = wpool.tile([128, F], F32, tag="bvbc")

            # ---- input DMAs. Tiny bias/constant transfers go first (the
            # kT bias-add gates the whole pipeline), then in gating order:
            # wk, kt half 0, q chunk 0, wq, kt half 1, wv, vt, wo.
            nc.sync.dma_start(bq_sb[:], d_bq.rearrange("(f p) -> p f", p=128))
            nc.sync.dma_start(bk_sb[:], d_bk.rearrange("(f p) -> p f", p=128))
            nc.sync.dma_start(bv_sb[:], d_bv.rearrange("(a f) -> a f", a=1))
            nc.sync.dma_start(ones_sb[:], d_ones[:])
            for m in range(8):
                nc.sync.dma_start(wk_sb[m][:], d_wk[128 * m:128 * (m + 1), :])
            # kt as [128, 1024] halves: kt_in[h][m] covers tokens 1024h..
            kt_in = [[ktin.tile([128, 1024], BF16, tag="ktin", name=f"kt{h}_{m}")
                      for m in range(8)] for h in range(2)]
            for m in range(8):
                nc.sync.dma_start(
                    kt_in[0][m][:], d_kt[128 * m:128 * (m + 1), 0:1024])

            def load_q_chunk(n):
                chs = []
                for m in range(8):
                    ch = qtin.tile([128, 512], BF16, tag="qtin")
                    nc.sync.dma_start(
                        ch[:], d_qt[128 * m:128 * (m + 1), 512 * n:512 * (n + 1)]
                    )
                    chs.append(ch)
                return chs

            qchs = {0: load_q_chunk(0)}
            for m in range(8):
                nc.sync.dma_start(wq_sb[m][:], d_wq[128 * m:128 * (m + 1), :])
            for m in range(8):
                nc.sync.dma_start(wv_sb[m][:], d_wv[128 * m:128 * (m + 1), :])
            # vt as [128, 512] quarters: vt_in[g][m] covers tokens 512g..
            # The pool holds 16 tiles, so quarters 2/3 reuse quarter 0/1
            # buffers; their DMAs are emitted from inside group (0,0) after
            # the v pieces that read the overwritten quarter.
            vt_in = [[None] * 8 for _ in range(4)]

            def load_vt_quarter(g):
                for m in range(8):
                    t = vtin.tile([128, 512], BF16, tag="vtin", name=f"vt{g}_{m}")
                    vt_in[g][m] = t
                    nc.sync.dma_start(
                        t[:], d_vt[128 * m:128 * (m + 1), 512 * g:512 * (g + 1)]
                    )

            load_vt_quarter(0)
            load_vt_quarter(1)
            for m in range(8):
                nc.sync.dma_start(
                    kt_in[1][m][:], d_kt[128 * m:128 * (m + 1), 1024:2048])
            load_vt_quarter(2)
            for f in range(4):
                nc.sync.dma_start(wo_sb[f][:], d_wo[128 * f:128 * (f + 1), :])

            # dummy exp: triggers the ~2.7us ACT table load while input
            # DMAs are still in flight
            scr_sb = wpool.tile([1, 128], BF16, tag="scr")
            nc.scalar.activation(scr_sb[:], ones_sb[0:1, :], EXP)

            # bv broadcast over partitions via K=1 matmul
            psbv = ps_misc.tile([128, 512], F32, tag="ps", name="psbv")
            nc.tensor.matmul(psbv[:], ones_sb[0:1, :], bv_sb[0:1, :])
            nc.vector.tensor_copy(bvbc_sb[:], psbv[:])

            # resident softmax-denominator staging: rows 0/32 hold the pair's
            # denominators, the rest stay 1.0 so the reciprocal is benign
            dn_sb = wpool.tile([65, 512], F32, tag="dn")
            rcd_sb = wpool.tile([65, 512], F32R, tag="rcd")
            nc.vector.memset(dn_sb[:], 1.0)



            # ---- projection pieces ------------------------------------
            def kt_piece(f, n, pool=None):
                h, c = n // 2, n % 2
                accp = (pool or ps_proj).tile([128, 512], F32, tag="po", name="acck")
                for m in range(8):
                    nc.tensor.matmul(
                        accp[:],
                        wk_sb[m][:, 128 * f:128 * (f + 1)],
                        kt_in[h][m][:, 512 * c:512 * (c + 1)],
                        start=(m == 0),
                        stop=(m == 7),
                    )
                with nc.allow_low_precision(reason="bf16 kT store"):
                    nc.vector.tensor_scalar_add(
                        kT_sb[f][:, 512 * n:512 * (n + 1)],
                        accp[:],
                        bk_sb[:, f:f + 1],
                    )

            def q_piece(n, f, pool=None, m_range=None, accp=None):
                if accp is None:
                    accp = (pool or ps_proj).tile([128, 512], F32, tag="po", name="accq")
                for m in (m_range or range(8)):
                    nc.tensor.matmul(
                        accp[:],
                        wq_sb[m][:, 128 * f:128 * (f + 1)],
                        qchs[n][m][:],
                        start=(m == 0),
                        stop=(m == 7),
                    )
                if m == 7:
                    with nc.allow_low_precision(reason="bf16 qT store"):
                        nc.vector.tensor_scalar_add(
                            qT_sb[f][:, 512 * n:512 * (n + 1)],
                            accp[:],
                            bq_sb[:, f:f + 1],
                        )
                return accp

            def q_piece_halves(n, f):
                st = {}

                def h1():
                    st["acc"] = q_piece(n, f, m_range=range(4))

                def h2():
                    q_piece(n, f, m_range=range(4, 8), accp=st["acc"])
                return [h1, h2]

            def v_piece(t, pool=None):
                g, c = t // 4, t % 4
                acc = (pool or ps_proj).tile([128, 512], F32, tag="po", name="accv")
                for m in range(8):
                    nc.tensor.matmul(
                        acc[:],
                        vt_in[g][m][:, 128 * c:128 * (c + 1)],
                        wv_sb[m][:],
                        start=(m == 0),
                        stop=(m == 7),
                    )
                v3 = v_sb[t][:].rearrange("p (h e) -> p h e", e=HD + 1)
                nc.vector.memset(v3[:, :, HD:HD + 1], 1.0)
                nc.vector.tensor_add(
                    v3[:, :, 0:HD],
                    acc[:].rearrange("p (h e) -> p h e", e=HD),
                    bvbc_sb[:].rearrange("p (h e) -> p h e", e=HD),
                )

            # ---- output projection ------------------------------------
            wo_pending = []

            def emit_wo(count=1):
                for _ in range(count):
                    if not wo_pending:
                        return
                    h1, h2 = wo_halves()
                    h1()
                    h2()

            def wo_halves():
                st = {}

                def h1():
                    if not wo_pending:
                        st["skip"] = True
                        return
                    st["ttj"] = wo_pending.pop(0)
                    tt, j = st["ttj"]
                    pw = ps_misc.tile([128, 512], F32, tag="ps", name="pw")
                    st["pw"] = pw
                    for f in range(2):
                        nc.tensor.matmul(
                            pw[:], oT_sb[f][:, 128 * tt:128 * (tt + 1)],
                            wo_sb[f][:, 512 * j:512 * (j + 1)],
                            start=(f == 0), stop=False,
                        )

                def h2():
                    if st.get("skip"):
                        return
                    tt, j = st["ttj"]
                    pw = st["pw"]
                    for f in range(2, 4):
                        nc.tensor.matmul(
                            pw[:], oT_sb[f][:, 128 * tt:128 * (tt + 1)],
                            wo_sb[f][:, 512 * j:512 * (j + 1)],
                            start=False, stop=(f == 3),
                        )
                    ost = outst.tile([128, 512], F32, tag="outst")
                    nc.vector.tensor_copy(ost[:], pw[:])
                    nc.sync.dma_start(
                        d_part[128 * tt:128 * (tt + 1), 512 * j:512 * (j + 1)],
                        ost[:])
                return h1, h2

            # ---- softmax normalization (per head pair) -----------------
            # phase1: evacuate both PV accumulators and gather the two
            #   denominator rows into one [2, 512] tile (frees PSUM early)
            # phase2: one fast approximate reciprocal for both heads
            # phase3 (per head): PE-broadcast the reciprocal row, multiply
            def norm_phase1(poA, poB):
                ocA = ocp.tile([65, 512], F32, tag="oc", name="ocA")
                ocB = ocp.tile([65, 512], F32, tag="oc", name="ocB")
                nc.vector.tensor_copy(ocA[:], poA[0:65, :])
                nc.vector.tensor_copy(ocB[:], poB[0:65, :])
                # both denominator rows into one resident tile at 32-aligned
                # partition bases so a single reciprocal covers the pair
                nc.vector.tensor_copy(dn_sb[0:1, :], ocA[64:65, :])
                nc.vector.tensor_copy(dn_sb[64:65, :], ocB[64:65, :])
                return [ocA, ocB]

            def norm_phase2(st):
                # one iterative-divide pass handles both heads (cost is
                # free-dim-bound; rows 1..31 hold 1.0 filler)
                with nc.allow_low_precision(reason="f32r recip of denom"):
                    nc.vector.reciprocal(rcd_sb[:], dn_sb[:])
                return rcd_sb

            def norm_phase3(st, rc, i, p, qsl):
                r0 = 64 * i
                pb = ps_misc.tile([128, 512], F32, tag="ps", name="pb")
                nc.tensor.matmul(pb[0:64, :], ones_sb[r0:r0 + 1, 0:64],
                                 rc[r0:r0 + 1, :], tile_position=(r0, 0))
                with nc.allow_low_precision(reason="bf16 normalized out"):
                    nc.vector.tensor_mul(
                        oT_sb[p][r0:r0 + 64, qsl], st[i][0:64, :], pb[0:64, :]
                    )

            # ---- attention group --------------------------------------
            # deadline[m]: closures run at the START of step m (before the
            # scores/PV emission — used for chunk 0's kT/v streaming).
            # fillers: one closure popped per step after scores/PV.
            prev_group = []   # [(poA, poB, p, qsl)] pending normalization

            def attn_group(n, p, fillers, deadline=None, lookahead=2):
                qsl = slice(512 * n, 512 * (n + 1))
                poA = ps_pv.tile([128, 512], F32, tag="po")
                poB = ps_pv.tile([128, 512], F32, tag="po")
                norm_st = {}

                def sc_emit(m):
                    scp = ps_sc.tile([128, 1024], F32, tag="sc")
                    ksl = slice(128 * m, 128 * (m + 1))
                    nc.tensor.matmul(
                        scp[:, 0:512], kT_sb[p][0:64, ksl], qT_sb[p][0:64, qsl],
                        tile_position=(0, 0),
                    )
                    nc.tensor.matmul(
                        scp[:, 512:1024], kT_sb[p][64:128, ksl],
                        qT_sb[p][64:128, qsl], tile_position=(64, 0),
                    )
                    ex = exch.tile([128, 1024], BF16, tag="ex")
                    nc.scalar.activation(ex[:], scp[:], EXP, scale=SCALE)
                    return ex

                exs = {j: sc_emit(j) for j in range(lookahead)}
                for m in range(KT):
                    # scores go first so ACT never waits behind filler work
                    if m + lookahead < KT:
                        exs[m + lookahead] = sc_emit(m + lookahead)
                    # previous group's normalization, early in this group so
                    # the PV accumulator WAR edge resolves before pv(0/1)
                    if prev_group:
                        if m == 0:
                            norm_st["st"] = norm_phase1(*prev_group[0][:2])
                        elif m == 2:
                            norm_st["rc"] = norm_phase2(norm_st["st"])
                        elif m in (4, 6):
                            _, _, p_, qsl_ = prev_group[0]
                            norm_phase3(norm_st["st"], norm_st["rc"],
                                        (m - 4) // 2, p_, qsl_)
                            if m == 6:
                                prev_group.pop(0)
                    if deadline and m in deadline:
                        for fn in deadline[m]:
                            fn()
                    ex = exs.pop(m)
                    nc.tensor.matmul(
                        poA[0:65, :], v_sb[m][:, 130 * p:130 * p + 65],
                        ex[:, 0:512], start=(m == 0), stop=(m == KT - 1),
                    )
                    nc.tensor.matmul(
                        poB[0:65, :], v_sb[m][:, 130 * p + 65:130 * p + 130],
                        ex[:, 512:1024], start=(m == 0), stop=(m == KT - 1),
                    )
                    if fillers:
                        fillers.pop(0)()
                prev_group.append((poA, poB, p, qsl))

            def drain_norms():
                while prev_group:
                    poA, poB, p_, qsl_ = prev_group.pop(0)
                    st = norm_phase1(poA, poB)
                    rc = norm_phase2(st)
                    norm_phase3(st, rc, 0, p_, qsl_)
                    norm_phase3(st, rc, 1, p_, qsl_)

            # ---- prologue: minimum to start group (0, 0). Projection
            # pieces ping-pong the two (still idle) PV banks. kt(0,1) is
            # needed by the 6-deep score prefetch at group start.
            kt_piece(0, 0, pool=ps_pv)
            kt_piece(0, 1, pool=ps_pv)
            q_piece(0, 0, pool=ps_pv)

            # ---- chunk 0: group (0,0) runs with a 6-deep score prefetch so
            # the ACT engine has work while the first vt DMAs land. v pieces
            # and kt(0,*) stream in as deadlines; kt f=1..3 / q(0,*) pieces
            # are fillers ordered so each lands a group ahead of its reader.
            fillq = [lambda: kt_piece(1, 0), lambda: kt_piece(2, 0),
                     lambda: kt_piece(3, 0)]
            for n4 in (1, 2, 3):
                fillq.append(lambda n=n4: q_piece(0, n))
                for f4 in (1, 2, 3):
                    fillq.append(lambda f=f4, n=n4: kt_piece(f, n))
            dl0 = {0: [lambda: v_piece(0), lambda: v_piece(1)],
                   1: [lambda: kt_piece(0, 2)],
                   5: [lambda: kt_piece(0, 3)]}
            # quarter 2/3 vt DMAs go after the last v piece reading the
            # quarter whose buffers they overwrite (v(3) at m=2, v(7) at m=6)
            dl0[3] = [lambda: load_vt_quarter(3)]
            for t in range(2, KT):
                dl0.setdefault(t - 1, []).append(lambda t=t: v_piece(t))
            attn_group(0, 0, fillq, deadline=dl0, lookahead=6)
            fillq.append(lambda: qchs.update({1: load_q_chunk(1)}))
            attn_group(0, 1, fillq)
            for p4 in range(4):
                fillq.extend(q_piece_halves(1, p4))
            attn_group(0, 2, fillq)
            attn_group(0, 3, fillq)
            for t in range(4):
                for j in range(2):
                    wo_pending.append((t, j))

            # ---- chunks 1-3: ACT-bound steady state --------------------
            for n in range(1, NQT):
                for p in range(NP):
                    if n < 3:
                        if p == 0:
                            fillq.append(
                                lambda nn=n: qchs.update({nn + 1: load_q_chunk(nn + 1)}))
                        fillq.extend(q_piece_halves(n + 1, p))
                    # wo reads oT rows the previous pair's normalization
                    # (slots 4/6) writes — keep it after those slots
                    wa1, wa2 = wo_halves()
                    wb1, wb2 = wo_halves()
                    wc1, wc2 = wo_halves()
                    attn_group(n, p, fillq,
                               deadline={8: [wa1], 9: [wa2], 11: [wb1],
                                         12: [wb2], 14: [wc1], 15: [wc2]})
                for t in range(4):
                    for j in range(2):
                        wo_pending.append((4 * n + t, j))

            # ---- tail: scores PSUM is idle now, so wo accumulators
            # double-buffer through ps_sc halves instead of the single
            # ps_misc bank (which would serialize matmuls behind copies)
            while fillq:
                fillq.pop(0)()
            drain_norms()
            cur = {}

            def tail_pw(i):
                if i % 2 == 0:
                    cur["t"] = ps_sc.tile([128, 1024], F32, tag="sc", name="pwt")
                return cur["t"][:, 512 * (i % 2):512 * (i % 2 + 1)]

            for i in range(len(wo_pending)):
                tt, j = wo_pending.pop(0)
                tsl = slice(128 * tt, 128 * (tt + 1))
                pw = tail_pw(i)
                for f in range(4):
                    nc.tensor.matmul(
                        pw[:], oT_sb[f][:, tsl],
                        wo_sb[f][:, 512 * j:512 * (j + 1)],
                        start=(f == 0), stop=(f == 3),
                    )
                ost = outst.tile([128, 512], F32, tag="outst")
                nc.vector.tensor_copy(ost[:], pw[:])
                nc.sync.dma_start(d_part[tsl, 512 * j:512 * (j + 1)], ost[:])

    _split_multiwaits(nc)
    return nc


_PROGRAM = None


def _get_program():
    global _PROGRAM
    if _PROGRAM is None:
        _PROGRAM = build_program()
    return _PROGRAM


def make_in_maps(Q, K, V, Wq, bq, Wk, bk, Wv, bv, Wo, bo):
    import ml_dtypes
    bf = lambda x: np.asarray(x, dtype=np.float32).astype(ml_dtypes.bfloat16)
    f32 = lambda x: np.asarray(x, dtype=np.float32)
    Q, K, V = bf(Q), bf(K), bf(V)
    Wq, Wk, Wv, Wo = bf(Wq), bf(Wk), bf(Wv), bf(Wo)
    bq, bk, bv = f32(bq), f32(bk), f32(bv)
    ones = np.ones((65, 128), np.float32)
    in_maps = []
    for c in range(N_CORES):
        b, hh = c // 2, c % 2
        fs = slice(F * hh, F * (hh + 1))
        in_maps.append({
            "qt": np.ascontiguousarray(Q[b].T),
            "kt": np.ascontiguousarray(K[b].T),
            "vt": np.ascontiguousarray(V[b].T),
            "wq": np.ascontiguousarray(Wq[:, fs]),
            "wk": np.ascontiguousarray(Wk[:, fs]),
            "wv": np.ascontiguousarray(Wv[:, fs]),
            "wo": np.ascontiguousarray(Wo[fs, :]),
            "bq": np.ascontiguousarray(bq[fs]),
            "bk": np.ascontiguousarray(bk[fs]),
            "bv": np.ascontiguousarray(bv[fs]),
            "ones": ones,
        })
    return in_maps


def kernel(Q, K, V, Wq, bq, Wk, bk, Wv, bv, Wo, bo, _trace=False, _trace_kwargs=None):
    nc = _get_program()
    in_maps = make_in_maps(Q, K, V, Wq, bq, Wk, bk, Wv, bv, Wo, bo)
    res = run_bass_kernel_spmd(
        nc, in_maps, core_ids=list(range(N_CORES)),
        trace=_trace, **(_trace_kwargs or {}),
    )
    parts = [r["part"] for r in res.results]
    out = np.stack([parts[2 * b] + parts[2 * b + 1] for b in range(B)])
    out += np.asarray(bo, dtype=np.float32)[None, None, :]
    if _trace:
        return out, res
    return out


# revision 36
# speedup vs baseline: 1.0130x; 1.0130x over previous
"""Multi-head attention (B=4, S=2048, D=1024, H=16) on 8 trn2 NeuronCores.

Sharding: (batch, head-half) -> one core each. Core c handles batch c//2 and
heads (c%2)*8 .. (c%2)*8+7 (feature columns (c%2)*512 .. +512 of the QKV
projections, rows of Wo). Each core computes its 8 heads' attention and a
partial output projection; the host sums the two partials per batch and adds
the output bias.

Device layout per core (S=2048 tokens, F=512 local features, hd=64):
  - inputs Q/K/V arrive host-transposed as [1024, 2048] so the d_model
    contraction sits on SBUF partitions,
  - q^T/k^T are produced feature-major ([512, 2048]) via lhsT=W, rhs=X^T,
  - scores are computed transposed (S^T[k, q]) so the P@V matmul can use v
    in natural [token, feature] layout as the stationary operand,
  - softmax: exp on the ACT engine with the 1/8 scale folded in; the
    denominator comes from an all-ones 65th column appended to v; the
    normalization multiplies o'^T rows by a PE-broadcast reciprocal row.

Schedule (v2): the kernel is ACT-bound in steady state (33.5M exps/core @
153.6 G/s ~= 220us floor), so attention group (0,0) starts as soon as the
kT tiles covering its first scores plus q(0,0) exist (~12us in, vs a ~150us
serial projection phase in v1). All remaining projection work (kT pieces,
v tiles, later q chunks) chases into the attention stream as deadline-driven
or filler work; chunks 1-3 are ACT-bound with PE ~85% busy. Softmax
denominators are gathered per head pair and inverted with the fast
approximate reciprocal (51 ULP) instead of an 8-cycle/elem DVE reciprocal
on a broadcast tile. Matmuls run in bf16 (fp32 PSUM accumulation).

PSUM banks: 4 scores (2 double-buffered [128,1024] tiles) + 2 PV
accumulators + 1 projection accumulator + 1 misc (broadcasts / Wo) = 8.
"""
import numpy as np

import concourse.bass as bass
import concourse.tile as tile
from concourse import mybir
from concourse.bass_utils import run_bass_kernel_spmd

F32 = mybir.dt.float32
F32R = mybir.dt.float32r
BF16 = mybir.dt.bfloat16
EXP = mybir.ActivationFunctionType.Exp

B, S, DM, H_TOT = 4, 2048, 1024, 16
F = 512          # features per core (8 heads x 64)
HD = 64          # head dim
NH = 8           # heads per core
NP = 4           # head pairs per core
KT = 16          # k tiles of 128
NQT = 4          # q chunks of 512
SCALE = 0.125    # 1/sqrt(64)
N_CORES = 8

_WAIT_CAP = {"InstEventSemaphore": 2}


def _split_multiwaits(nc):
    """This walrus build accepts 1 sync-wait per instruction (2 on
    EventSemaphore); spread extras over same-engine NOPs placed before."""
    n_spill = 0
    for f in nc.m.functions:
        for bb in f.blocks:
            new = []
            changed = False
            for inst in bb.instructions:
                si = inst.sync_info
                cap = _WAIT_CAP.get(type(inst).__name__, 1)
                if si is not None and len(si.on_wait) > cap:
                    extra = list(si.on_wait[: len(si.on_wait) - cap])
                    del si.on_wait[: len(si.on_wait) - cap]
                    for w in extra:
                        n_spill += 1
                        nop = mybir.InstNoOp(name=f"I-wspill-{n_spill}-{inst.name}")
                        nop.engine = inst.engine
                        nop.sync_info = mybir.SyncInfo(on_wait=[w], on_update=[])
                        new.append(nop)
                    changed = True
                new.append(inst)
            if changed:
                bb.instructions[:] = new
    return n_spill


def build_program():
    nc = bass.Bass("TRN2", target_bir_lowering=False, debug=False, num_devices=1)

    d_qt = nc.dram_tensor("qt", [DM, S], BF16, kind="ExternalInput").ap()
    d_kt = nc.dram_tensor("kt", [DM, S], BF16, kind="ExternalInput").ap()
    d_vt = nc.dram_tensor("vt", [DM, S], BF16, kind="ExternalInput").ap()
    d_wq = nc.dram_tensor("wq", [DM, F], BF16, kind="ExternalInput").ap()
    d_wk = nc.dram_tensor("wk", [DM, F], BF16, kind="ExternalInput").ap()
    d_wv = nc.dram_tensor("wv", [DM, F], BF16, kind="ExternalInput").ap()
    d_wo = nc.dram_tensor("wo", [F, DM], BF16, kind="ExternalInput").ap()
    d_bq = nc.dram_tensor("bq", [F], F32, kind="ExternalInput").ap()
    d_bk = nc.dram_tensor("bk", [F], F32, kind="ExternalInput").ap()
    d_bv = nc.dram_tensor("bv", [F], F32R, kind="ExternalInput").ap()
    d_ones = nc.dram_tensor("ones", [65, 128], F32R, kind="ExternalInput").ap()
    d_part = nc.dram_tensor("part", [S, DM], F32, kind="ExternalOutput").ap()

    with tile.TileContext(nc) as tc:
        with (
            tc.tile_pool(name="wpool", bufs=1) as wpool,
            tc.tile_pool(name="big", bufs=1) as big,
            tc.tile_pool(name="ktin", bufs=16) as ktin,
            tc.tile_pool(name="qtin", bufs=10) as qtin,
            tc.tile_pool(name="vtin", bufs=16) as vtin,
            tc.tile_pool(name="exch", bufs=8) as exch,
            tc.tile_pool(name="outst", bufs=2) as outst,
            tc.tile_pool(name="ocp", bufs=5) as ocp,
            tc.tile_pool(name="ps_sc", bufs=2, space="PSUM") as ps_sc,
            tc.tile_pool(name="ps_pv", bufs=2, space="PSUM") as ps_pv,
            tc.tile_pool(name="ps_proj", bufs=1, space="PSUM") as ps_proj,
            tc.tile_pool(name="ps_misc", bufs=1, space="PSUM") as ps_misc,
        ):
            # ---- resident tiles
            wq_sb = [wpool.tile([128, F], BF16, tag=f"wq{m}", name=f"wq{m}") for m in range(8)]
            wk_sb = [wpool.tile([128, F], BF16, tag=f"wk{m}", name=f"wk{m}") for m in range(8)]
            wv_sb = [wpool.tile([128, F], BF16, tag=f"wv{m}", name=f"wv{m}") for m in range(8)]
            wo_sb = [wpool.tile([128, DM], BF16, tag=f"wo{f}", name=f"wo{f}") for f in range(4)]
            qT_sb = [big.tile([128, S], BF16, tag=f"qT{f}", name=f"qT{f}") for f in range(4)]
            kT_sb = [big.tile([128, S], BF16, tag=f"kT{f}", name=f"kT{f}") for f in range(4)]
            oT_sb = [big.tile([128, S], BF16, tag=f"oT{f}", name=f"oT{f}") for f in range(4)]
            v_sb = [big.tile([128, NH * (HD + 1)], BF16, tag=f"v{t}", name=f"v{t}") for t in range(KT)]
            bq_sb = wpool.tile([128, 4], F32, tag="bq")
            bk_sb = wpool.tile([128, 4], F32, tag="bk")
            bv_sb = wpool.tile([1, F], F32R, tag="bv")
            ones_sb = wpool.tile([65, 128], F32R, tag="ones")
            bvbc_sb = wpool.tile([128, F], F32, tag="bvbc")

            # ---- input DMAs. Tiny bias/constant transfers go first (the
            # kT bias-add gates the whole pipeline), then in gating order:
            # wk, kt half 0, q chunk 0, wq, kt half 1, wv, vt, wo.
            nc.sync.dma_start(bq_sb[:], d_bq.rearrange("(f p) -> p f", p=128))
            nc.sync.dma_start(bk_sb[:], d_bk.rearrange("(f p) -> p f", p=128))
            nc.sync.dma_start(bv_sb[:], d_bv.rearrange("(a f) -> a f", a=1))
            nc.sync.dma_start(ones_sb[:], d_ones[:])
            for m in range(8):
                nc.sync.dma_start(wk_sb[m][:], d_wk[128 * m:128 * (m + 1), :])
            # kt as [128, 1024] halves: kt_in[h][m] covers tokens 1024h..
            kt_in = [[ktin.tile([128, 1024], BF16, tag="ktin", name=f"kt{h}_{m}")
                      for m in range(8)] for h in range(2)]
            for m in range(8):
                nc.sync.dma_start(
                    kt_in[0][m][:], d_kt[128 * m:128 * (m + 1), 0:1024])

            def load_q_chunk(n):
                chs = []
                for m in range(8):
                    ch = qtin.tile([128, 512], BF16, tag="qtin")
                    nc.sync.dma_start(
                        ch[:], d_qt[128 * m:128 * (m + 1), 512 * n:512 * (n + 1)]
                    )
                    chs.append(ch)
                return chs

            qchs = {0: load_q_chunk(0)}
            for m in range(8):
                nc.sync.dma_start(wq_sb[m][:], d_wq[128 * m:128 * (m + 1), :])
            for m in range(8):
                nc.sync.dma_start(
                    kt_in[1][m][:], d_kt[128 * m:128 * (m + 1), 1024:2048])
            for m in range(8):
                nc.sync.dma_start(wv_sb[m][:], d_wv[128 * m:128 * (m + 1), :])
            # vt as [128, 512] quarters: vt_in[g][m] covers tokens 512g..
            # The pool holds 16 tiles, so quarters 2/3 reuse quarter 0/1
            # buffers; their DMAs are emitted from inside group (0,0) after
            # the v pieces that read the overwritten quarter.
            vt_in = [[None] * 8 for _ in range(4)]

            def load_vt_quarter(g):
                for m in range(8):
                    t = vtin.tile([128, 512], BF16, tag="vtin", name=f"vt{g}_{m}")
                    vt_in[g][m] = t
                    nc.sync.dma_start(
                        t[:], d_vt[128 * m:128 * (m + 1), 512 * g:512 * (g + 1)]
                    )

            load_vt_quarter(0)
            load_vt_quarter(1)
            for f in range(4):
                nc.sync.dma_start(wo_sb[f][:], d_wo[128 * f:128 * (f + 1), :])

            # dummy exp: triggers the ~2.7us ACT table load while input
            # DMAs are still in flight
            scr_sb = wpool.tile([1, 128], BF16, tag="scr")
            nc.scalar.activation(scr_sb[:], ones_sb[0:1, :], EXP)

            # bv broadcast over partitions via K=1 matmul
            psbv = ps_misc.tile([128, 512], F32, tag="ps", name="psbv")
            nc.tensor.matmul(psbv[:], ones_sb[0:1, :], bv_sb[0:1, :])
            nc.vector.tensor_copy(bvbc_sb[:], psbv[:])

            # resident softmax-denominator staging: rows 0/32 hold the pair's
            # denominators, the rest stay 1.0 so the reciprocal is benign
            dn_sb = wpool.tile([65, 512], F32, tag="dn")
            rcd_sb = wpool.tile([65, 512], F32R, tag="rcd")
            nc.vector.memset(dn_sb[:], 1.0)



            # ---- projection pieces ------------------------------------
            def kt_piece(f, n, pool=None):
                h, c = n // 2, n % 2
                accp = (pool or ps_proj).tile([128, 512], F32, tag="po", name="acck")
                for m in range(8):
                    nc.tensor.matmul(
                        accp[:],
                        wk_sb[m][:, 128 * f:128 * (f + 1)],
                        kt_in[h][m][:, 512 * c:512 * (c + 1)],
                        start=(m == 0),
                        stop=(m == 7),
                    )
                with nc.allow_low_precision(reason="bf16 kT store"):
                    nc.vector.tensor_scalar_add(
                        kT_sb[f][:, 512 * n:512 * (n + 1)],
                        accp[:],
                        bk_sb[:, f:f + 1],
                    )

            def q_piece(n, f, pool=None, m_range=None, accp=None):
                if accp is None:
                    accp = (pool or ps_proj).tile([128, 512], F32, tag="po", name="accq")
                for m in (m_range or range(8)):
                    nc.tensor.matmul(
                        accp[:],
                        wq_sb[m][:, 128 * f:128 * (f + 1)],
                        qchs[n][m][:],
                        start=(m == 0),
                        stop=(m == 7),
                    )
                if m == 7:
                    with nc.allow_low_precision(reason="bf16 qT store"):
                        nc.vector.tensor_scalar_add(
                            qT_sb[f][:, 512 * n:512 * (n + 1)],
                            accp[:],
                            bq_sb[:, f:f + 1],
                        )
                return accp

            def q_piece_halves(n, f):
                st = {}

                def h1():
                    st["acc"] = q_piece(n, f, m_range=range(4))

                def h2():
                    q_piece(n, f, m_range=range(4, 8), accp=st["acc"])
                return [h1, h2]

            def v_piece(t, pool=None):
                g, c = t // 4, t % 4
                acc = (pool or ps_proj).tile([128, 512], F32, tag="po", name="accv")
                for m in range(8):
                    nc.tensor.matmul(
                        acc[:],
                        vt_in[g][m][:, 128 * c:128 * (c + 1)],
                        wv_sb[m][:],
                        start=(m == 0),
                        stop=(m == 7),
                    )
                v3 = v_sb[t][:].rearrange("p (h e) -> p h e", e=HD + 1)
                nc.vector.memset(v3[:, :, HD:HD + 1], 1.0)
                nc.vector.tensor_add(
                    v3[:, :, 0:HD],
                    acc[:].rearrange("p (h e) -> p h e", e=HD),
                    bvbc_sb[:].rearrange("p (h e) -> p h e", e=HD),
                )

            # ---- output projection ------------------------------------
            wo_pending = []

            def emit_wo(count=1):
                for _ in range(count):
                    if not wo_pending:
                        return
                    h1, h2 = wo_halves()
                    h1()
                    h2()

            def wo_halves():
                st = {}

                def h1():
                    if not wo_pending:
                        st["skip"] = True
                        return
                    st["ttj"] = wo_pending.pop(0)
                    tt, j = st["ttj"]
                    pw = ps_misc.tile([128, 512], F32, tag="ps", name="pw")
                    st["pw"] = pw
                    for f in range(2):
                        nc.tensor.matmul(
                            pw[:], oT_sb[f][:, 128 * tt:128 * (tt + 1)],
                            wo_sb[f][:, 512 * j:512 * (j + 1)],
                            start=(f == 0), stop=False,
                        )

                def h2():
                    if st.get("skip"):
                        return
                    tt, j = st["ttj"]
                    pw = st["pw"]
                    for f in range(2, 4):
                        nc.tensor.matmul(
                            pw[:], oT_sb[f][:, 128 * tt:128 * (tt + 1)],
                            wo_sb[f][:, 512 * j:512 * (j + 1)],
                            start=False, stop=(f == 3),
                        )
                    ost = outst.tile([128, 512], F32, tag="outst")
                    nc.vector.tensor_copy(ost[:], pw[:])
                    nc.sync.dma_start(
                        d_part[128 * tt:128 * (tt + 1), 512 * j:512 * (j + 1)],
                        ost[:])
                return h1, h2

            # ---- softmax normalization (per head pair) -----------------
            # phase1: evacuate both PV accumulators and gather the two
            #   denominator rows into one [2, 512] tile (frees PSUM early)
            # phase2: one fast approximate reciprocal for both heads
            # phase3 (per head): PE-broadcast the reciprocal row, multiply
            def norm_phase1(poA, poB):
                ocA = ocp.tile([65, 512], F32, tag="oc", name="ocA")
                ocB = ocp.tile([65, 512], F32, tag="oc", name="ocB")
                nc.vector.tensor_copy(ocA[:], poA[0:65, :])
                nc.vector.tensor_copy(ocB[:], poB[0:65, :])
                # both denominator rows into one resident tile at 32-aligned
                # partition bases so a single reciprocal covers the pair
                nc.vector.tensor_copy(dn_sb[0:1, :], ocA[64:65, :])
                nc.vector.tensor_copy(dn_sb[64:65, :], ocB[64:65, :])
                return [ocA, ocB]

            def norm_phase2(st):
                # one iterative-divide pass handles both heads (cost is
                # free-dim-bound; rows 1..31 hold 1.0 filler)
                with nc.allow_low_precision(reason="f32r recip of denom"):
                    nc.vector.reciprocal(rcd_sb[:], dn_sb[:])
                return rcd_sb

            def norm_phase3(st, rc, i, p, qsl):
                r0 = 64 * i
                pb = ps_misc.tile([128, 512], F32, tag="ps", name="pb")
                nc.tensor.matmul(pb[0:64, :], ones_sb[r0:r0 + 1, 0:64],
                                 rc[r0:r0 + 1, :], tile_position=(r0, 0))
                with nc.allow_low_precision(reason="bf16 normalized out"):
                    nc.vector.tensor_mul(
                        oT_sb[p][r0:r0 + 64, qsl], st[i][0:64, :], pb[0:64, :]
                    )

            # ---- attention group --------------------------------------
            # deadline[m]: closures run at the START of step m (before the
            # scores/PV emission — used for chunk 0's kT/v streaming).
            # fillers: one closure popped per step after scores/PV.
            prev_group = []   # [(poA, poB, p, qsl)] pending normalization

            def attn_group(n, p, fillers, deadline=None, lookahead=2):
                qsl = slice(512 * n, 512 * (n + 1))
                poA = ps_pv.tile([128, 512], F32, tag="po")
                poB = ps_pv.tile([128, 512], F32, tag="po")
                norm_st = {}

                def sc_emit(m):
                    scp = ps_sc.tile([128, 1024], F32, tag="sc")
                    ksl = slice(128 * m, 128 * (m + 1))
                    nc.tensor.matmul(
                        scp[:, 0:512], kT_sb[p][0:64, ksl], qT_sb[p][0:64, qsl],
                        tile_position=(0, 0),
                    )
                    nc.tensor.matmul(
                        scp[:, 512:1024], kT_sb[p][64:128, ksl],
                        qT_sb[p][64:128, qsl], tile_position=(64, 0),
                    )
                    ex = exch.tile([128, 1024], BF16, tag="ex")
                    nc.scalar.activation(ex[:], scp[:], EXP, scale=SCALE)
                    return ex

                exs = {j: sc_emit(j) for j in range(lookahead)}
                for m in range(KT):
                    # scores go first so ACT never waits behind filler work
                    if m + lookahead < KT:
                        exs[m + lookahead] = sc_emit(m + lookahead)
                    # previous group's normalization, early in this group so
                    # the PV accumulator WAR edge resolves before pv(0/1)
                    if prev_group:
                        if m == 0:
                            norm_st["st"] = norm_phase1(*prev_group[0][:2])
                        elif m == 2:
                            norm_st["rc"] = norm_phase2(norm_st["st"])
                        elif m in (4, 6):
                            _, _, p_, qsl_ = prev_group[0]
                            norm_phase3(norm_st["st"], norm_st["rc"],
                                        (m - 4) // 2, p_, qsl_)
                            if m == 6:
                                prev_group.pop(0)
                    if deadline and m in deadline:
                        for fn in deadline[m]:
                            fn()
                    ex = exs.pop(m)
                    nc.tensor.matmul(
                        poA[0:65, :], v_sb[m][:, 130 * p:130 * p + 65],
                        ex[:, 0:512], start=(m == 0), stop=(m == KT - 1),
                    )
                    nc.tensor.matmul(
                        poB[0:65, :], v_sb[m][:, 130 * p + 65:130 * p + 130],
                        ex[:, 512:1024], start=(m == 0), stop=(m == KT - 1),
                    )
                    if fillers:
                        fillers.pop(0)()
                prev_group.append((poA, poB, p, qsl))

            def drain_norms():
                while prev_group:
                    poA, poB, p_, qsl_ = prev_group.pop(0)
                    st = norm_phase1(poA, poB)
                    rc = norm_phase2(st)
                    norm_phase3(st, rc, 0, p_, qsl_)
                    norm_phase3(st, rc, 1, p_, qsl_)

            # ---- prologue: minimum to start group (0, 0). Projection
            # pieces ping-pong the two (still idle) PV banks. kt(0,1) is
            # needed by the 6-deep score prefetch at group start.
            kt_piece(0, 0, pool=ps_pv)
            kt_piece(0, 1, pool=ps_pv)
            q_piece(0, 0, pool=ps_pv)

            # ---- chunk 0: group (0,0) runs with a 6-deep score prefetch so
            # the ACT engine has work while the first vt DMAs land. v pieces
            # and kt(0,*) stream in as deadlines; kt f=1..3 / q(0,*) pieces
            # are fillers ordered so each lands a group ahead of its reader.
            fillq = [lambda: kt_piece(1, 0), lambda: kt_piece(2, 0),
                     lambda: kt_piece(3, 0)]
            for n4 in (1, 2, 3):
                fillq.append(lambda n=n4: q_piece(0, n))
                for f4 in (1, 2, 3):
                    fillq.append(lambda f=f4, n=n4: kt_piece(f, n))
            dl0 = {0: [lambda: v_piece(0), lambda: v_piece(1)],
                   1: [lambda: kt_piece(0, 2)],
                   5: [lambda: kt_piece(0, 3)]}
            # quarter 2/3 vt DMAs go after the last v piece reading the
            # quarter whose buffers they overwrite (v(3) at m=2, v(7) at m=6)
            dl0[3] = [lambda: load_vt_quarter(2)]
            dl0[7] = [lambda: load_vt_quarter(3)]
            for t in range(2, KT):
                dl0.setdefault(t - 1, []).append(lambda t=t: v_piece(t))
            attn_group(0, 0, fillq, deadline=dl0, lookahead=6)
            fillq.append(lambda: qchs.update({1: load_q_chunk(1)}))
            attn_group(0, 1, fillq)
            for p4 in range(4):
                fillq.extend(q_piece_halves(1, p4))
            attn_group(0, 2, fillq)
            attn_group(0, 3, fillq)
            for t in range(4):
                for j in range(2):
                    wo_pending.append((t, j))

            # ---- chunks 1-3: ACT-bound steady state --------------------
            for n in range(1, NQT):
                for p in range(NP):
                    if n < 3:
                        if p == 0:
                            fillq.append(
                                lambda nn=n: qchs.update({nn + 1: load_q_chunk(nn + 1)}))
                        fillq.extend(q_piece_halves(n + 1, p))
                    # wo reads oT rows the previous pair's normalization
                    # (slots 4/6) writes — keep it after those slots
                    wa1, wa2 = wo_halves()
                    wb1, wb2 = wo_halves()
                    wc1, wc2 = wo_halves()
                    attn_group(n, p, fillq,
                               deadline={8: [wa1], 9: [wa2], 11: [wb1],
                                         12: [wb2], 14: [wc1], 15: [wc2]})
                for t in range(4):
                    for j in range(2):
                        wo_pending.append((4 * n + t, j))

            # ---- tail: scores PSUM is idle now, so wo accumulators
            # double-buffer through ps_sc halves instead of the single
            # ps_misc bank (which would serialize matmuls behind copies)
            while fillq:
                fillq.pop(0)()
            drain_norms()
            cur = {}

            def tail_pw(i):
                if i % 2 == 0:
                    cur["t"] = ps_sc.tile([128, 1024], F32, tag="sc", name="pwt")
                return cur["t"][:, 512 * (i % 2):512 * (i % 2 + 1)]

            for i in range(len(wo_pending)):
                tt, j = wo_pending.pop(0)
                tsl = slice(128 * tt, 128 * (tt + 1))
                pw = tail_pw(i)
                for f in range(4):
                    nc.tensor.matmul(
                        pw[:], oT_sb[f][:, tsl],
                        wo_sb[f][:, 512 * j:512 * (j + 1)],
                        start=(f == 0), stop=(f == 3),
                    )
                ost = outst.tile([128, 512], F32, tag="outst")
                nc.vector.tensor_copy(ost[:], pw[:])
                nc.sync.dma_start(d_part[tsl, 512 * j:512 * (j + 1)], ost[:])

    _split_multiwaits(nc)
    return nc


_PROGRAM = None


def _get_program():
    global _PROGRAM
    if _PROGRAM is None:
        _PROGRAM = build_program()
    return _PROGRAM


def make_in_maps(Q, K, V, Wq, bq, Wk, bk, Wv, bv, Wo, bo):
    import ml_dtypes
    bf = lambda x: np.asarray(x, dtype=np.float32).astype(ml_dtypes.bfloat16)
    f32 = lambda x: np.asarray(x, dtype=np.float32)
    Q, K, V = bf(Q), bf(K), bf(V)
    Wq, Wk, Wv, Wo = bf(Wq), bf(Wk), bf(Wv), bf(Wo)
    bq, bk, bv = f32(bq), f32(bk), f32(bv)
    ones = np.ones((65, 128), np.float32)
    in_maps = []
    for c in range(N_CORES):
        b, hh = c // 2, c % 2
        fs = slice(F * hh, F * (hh + 1))
        in_maps.append({
            "qt": np.ascontiguousarray(Q[b].T),
            "kt": np.ascontiguousarray(K[b].T),
            "vt": np.ascontiguousarray(V[b].T),
            "wq": np.ascontiguousarray(Wq[:, fs]),
            "wk": np.ascontiguousarray(Wk[:, fs]),
            "wv": np.ascontiguousarray(Wv[:, fs]),
            "wo": np.ascontiguousarray(Wo[fs, :]),
            "bq": np.ascontiguousarray(bq[fs]),
            "bk": np.ascontiguousarray(bk[fs]),
            "bv": np.ascontiguousarray(bv[fs]),
            "ones": ones,
        })
    return in_maps


def kernel(Q, K, V, Wq, bq, Wk, bk, Wv, bv, Wo, bo, _trace=False, _trace_kwargs=None):
    nc = _get_program()
    in_maps = make_in_maps(Q, K, V, Wq, bq, Wk, bk, Wv, bv, Wo, bo)
    res = run_bass_kernel_spmd(
        nc, in_maps, core_ids=list(range(N_CORES)),
        trace=_trace, **(_trace_kwargs or {}),
    )
    parts = [r["part"] for r in res.results]
    out = np.stack([parts[2 * b] + parts[2 * b + 1] for b in range(B)])
    out += np.asarray(bo, dtype=np.float32)[None, None, :]
    if _trace:
        return out, res
    return out
